# revision 1
# baseline (speedup 1.0000x reference)
"""Multi-head self-attention with LoRA on 8 Trainium2 NeuronCores.

Sharding: core c -> (batch b = c//2, query-token-half = c%2).
Each core:
  - transposes its batch's x [2048, 2048] on the PE (fp32 exact)
  - computes qT for its 1024 query tokens, kT/vT for all 2048 batch tokens
    (K/V projection duplicated across the 2 cores of a batch — avoids all
    cross-core communication)
  - LoRA is folded in as one extra rank-16 accumulation matmul per output tile
  - attention per head: scores -> exp -> ones-matmul denominators -> attn@v
    (v re-transposed to natural layout on the PE per head)
  - O-projection for its 1024 tokens, output written transposed [D, 1024]
Host: input layout prep (slices/transposes only) and output assembly.
All heavy matmuls run as float32r (fp22 multiply, fp32 accumulate).
"""

import os
import numpy as np

import concourse.bacc as bacc
import concourse.mybir as mybir
import concourse.tile as tile
from concourse.bass_utils import run_bass_kernel_spmd

F32 = mybir.dt.float32
F32R = mybir.dt.float32r
AF = mybir.ActivationFunctionType

B, L, D = 4, 2048, 2048
H, HD, R = 16, 128, 16
SCALING = 0.5          # lora alpha / rank
SCALE = HD ** -0.5     # attention score scale
P = 128                # partitions
NT = D // P            # 16 tiles along feature dims
TT = L // P            # 16 tiles along token dim
QTOK = L // 2          # query tokens per core
CH = 512               # moving-dim chunk
NCORES = 8

_cache = {}


def _build():
    nc = bacc.Bacc()

    xb = nc.dram_tensor("xb", [L, D], F32, kind="ExternalInput")
    wT = {p: nc.dram_tensor(f"w{p}T", [D, D], F32, kind="ExternalInput") for p in "qkvo"}
    bias = {p: nc.dram_tensor(f"b{p}", [D], F32, kind="ExternalInput") for p in "qkvo"}
    aT = {p: nc.dram_tensor(f"a{p}T", [R, D], F32, kind="ExternalInput") for p in "qkvo"}
    bT = {p: nc.dram_tensor(f"bt{p}", [D, R], F32, kind="ExternalInput") for p in "qkvo"}
    yt = nc.dram_tensor("yt", [D, QTOK], F32, kind="ExternalOutput")

    ident_d = nc.inline_tensor(np.eye(P, dtype=np.float32), name="ident_d")
    ones_d = nc.inline_tensor(np.ones((P, P), dtype=np.float32), name="ones_d")

    def dma(out, in_, f32r=False):
        if f32r:
            nc.sync.dma_start(out=out.bitcast(F32R), in_=in_.bitcast(F32R))
        else:
            nc.sync.dma_start(out=out, in_=in_)

    def r(ap):
        return ap.bitcast(F32R)

    with tile.TileContext(nc) as tc:
        with (
            tc.tile_pool(name="consts", bufs=1) as consts,
            tc.tile_pool(name="dram", bufs=1, space="DRAM") as dpool,
        ):
            # ---- persistent constants ----
            ident = consts.tile([P, P], F32, tag="ident")
            dma(ident, ident_d[:, :])
            ones = consts.tile([P, P], F32, tag="ones")
            dma(ones, ones_d[:, :], f32r=True)

            # biases as [128, 4, 16] (per-partition scalar per (proj, dout tile))
            biasall = consts.tile([P, 4, NT], F32, tag="biasall")
            for p in "qkvo":
                dma(biasall[:, "qkvo".index(p), :],
                    bias[p][:].rearrange("(t p) -> p t", p=P))

            # LoRA B^T as [128, 4, 16, 16] f32r
            bTall = consts.tile([P, 4, NT, R], F32, tag="bTall")
            for p in "qkvo":
                dma(bTall[:, "qkvo".index(p), :, :],
                    bT[p][:, :].rearrange("(n p) r -> p n r", p=P), f32r=True)

            # z LoRA intermediates: [16, {q,k,v}, L] (q uses first QTOK cols)
            z3 = consts.tile([R, 3, L], F32, tag="z3")
            zo = consts.tile([R, QTOK], F32, tag="zo")

            # DRAM scratch
            qT_d = dpool.tile([D, QTOK], F32, tag="qT_d")
            kT_d = dpool.tile([D, L], F32, tag="kT_d")
            vT_d = dpool.tile([D, L], F32, tag="vT_d")
            ao_d = dpool.tile([D, QTOK], F32, tag="ao_d")

            # =============== Phase 1: transpose x -> xT (SBUF resident) =======
            with tc.tile_pool(name="xT", bufs=1) as xTpool:
                xT = xTpool.tile([P, NT, L], F32, tag="xT")  # [p, din_tile, tok]

                with (
                    tc.tile_pool(name="stage", bufs=2) as stage,
                    tc.tile_pool(name="pt", bufs=4, space="PSUM") as pt,
                ):
                    for ti in range(TT):
                        st = stage.tile([P, D], F32, tag="st")
                        dma(st, xb[ti * P:(ti + 1) * P, :])
                        for di in range(NT):
                            ps = pt.tile([P, P], F32, tag="pt")
                            nc.tensor.transpose(ps, st[:, di * P:(di + 1) * P], ident)
                            nc.vector.tensor_copy(
                                out=r(xT[:, di, ti * P:(ti + 1) * P]), in_=ps)

                # =============== Phase 2a: z = SCALING * (B @ xT) ============
                with tc.tile_pool(name="pz", bufs=2, space="PSUM") as pz:
                    for pi, (p, tokn) in enumerate((("q", QTOK), ("k", L), ("v", L))):
                        for c0 in range(0, tokn, CH):
                            ps = pz.tile([R, CH], F32, tag="pz")
                            for di in range(NT):
                                nc.tensor.matmul(ps, r(bTall[:, pi, di, :]),
                                                 r(xT[:, di, c0:c0 + CH]),
                                                 start=(di == 0), stop=(di == NT - 1))
                            nc.vector.tensor_scalar_mul(
                                r(z3[:, pi, c0:c0 + CH]), ps, SCALING)

                # =============== Phase 2b: qT / kT / vT projections ==========
                with (
                    tc.tile_pool(name="wqk", bufs=2) as wpool,
                    tc.tile_pool(name="aqk", bufs=1) as apool2,
                    tc.tile_pool(name="oqk", bufs=3) as opool,
                    tc.tile_pool(name="pqk", bufs=4, space="PSUM") as pp,
                ):
                    for pi, (p, tokn, dest) in enumerate(
                            (("q", QTOK, qT_d), ("k", L, kT_d), ("v", L, vT_d))):
                        at_sb = apool2.tile([R, D], F32, tag="aTqk")
                        dma(at_sb, aT[p][:, :], f32r=True)
                        for do in range(NT):
                            w_sb = wpool.tile([P, NT, P], F32, tag="wqk")
                            dma(w_sb, wT[p][:, do * P:(do + 1) * P]
                                .rearrange("(n p) f -> p n f", p=P), f32r=True)
                            for c0 in range(0, tokn, CH):
                                ps = pp.tile([P, CH], F32, tag="pqk")
                                for ki in range(NT):
                                    nc.tensor.matmul(ps, r(w_sb[:, ki, :]),
                                                     r(xT[:, ki, c0:c0 + CH]),
                                                     start=(ki == 0), stop=False)
                                nc.tensor.matmul(ps, r(at_sb[:, do * P:(do + 1) * P]),
                                                 r(z3[:, pi, c0:c0 + CH]),
                                                 start=False, stop=True)
                                o_sb = opool.tile([P, CH], F32, tag="oqk")
                                nc.vector.tensor_scalar_add(o_sb, ps,
                                                            biasall[:, pi, do:do + 1])
                                dma(dest[do * P:(do + 1) * P, c0:c0 + CH], o_sb)

            # =============== Phase 3: attention per head =====================
            with (
                tc.tile_pool(name="heads", bufs=2) as hpool,
                tc.tile_pool(name="vh", bufs=1) as vhpool,
                tc.tile_pool(name="ex", bufs=2) as expool,
                tc.tile_pool(name="att_sb", bufs=3) as asbpool,
                tc.tile_pool(name="ps_s", bufs=4, space="PSUM") as ps_spool,
                tc.tile_pool(name="ps_d", bufs=1, space="PSUM") as ps_dpool,
                tc.tile_pool(name="ps_r", bufs=1, space="PSUM") as ps_rpool,
                tc.tile_pool(name="ps_o", bufs=2, space="PSUM") as ps_opool,
            ):
                for hh in range(H):
                    kT_h = hpool.tile([P, L], F32, tag="kT")
                    dma(kT_h, kT_d[hh * P:(hh + 1) * P, :], f32r=True)
                    qT_h = hpool.tile([P, QTOK], F32, tag="qT")
                    dma(qT_h, qT_d[hh * P:(hh + 1) * P, :], f32r=True)
                    vT_h = hpool.tile([P, L], F32, tag="vT")
                    dma(vT_h, vT_d[hh * P:(hh + 1) * P, :])
                    # re-transpose v to natural [key, hd] layout on the PE
                    v_h = vhpool.tile([P, TT, P], F32, tag="v_h")
                    for kt in range(TT):
                        ps_t = ps_spool.tile([P, P], F32, tag="ps_s")
                        nc.tensor.transpose(ps_t, vT_h[:, kt * P:(kt + 1) * P], ident)
                        nc.vector.tensor_copy(out=r(v_h[:, kt, :]), in_=ps_t)

                    for c0 in range(0, QTOK, CH):
                        ex = expool.tile([P, TT, CH], F32, tag="ex")
                        for kt in range(TT):
                            ps_s = ps_spool.tile([P, CH], F32, tag="ps_s")
                            nc.tensor.matmul(ps_s, r(kT_h[:, kt * P:(kt + 1) * P]),
                                             r(qT_h[:, c0:c0 + CH]),
                                             start=True, stop=True)
                            nc.scalar.activation(r(ex[:, kt, :]), ps_s,
                                                 AF.Exp, scale=SCALE)
                        # denominators: ones.T @ ex summed over all key tiles
                        ps_d = ps_dpool.tile([1, CH], F32, tag="ps_d")
                        for kt in range(TT):
                            nc.tensor.matmul(ps_d, r(ones[:, 0:1]), r(ex[:, kt, :]),
                                             start=(kt == 0), stop=(kt == TT - 1))
                        d_sb = asbpool.tile([1, CH], F32, tag="dsb")
                        nc.vector.tensor_copy(out=r(d_sb), in_=ps_d)
                        # attn @ v
                        ps_o = ps_opool.tile([P, CH], F32, tag="ps_o")
                        for kt in range(TT):
                            nc.tensor.matmul(ps_o, r(v_h[:, kt, :]), r(ex[:, kt, :]),
                                             start=(kt == 0), stop=(kt == TT - 1))
                        # normalize: ao = ps_o * (1/denom) broadcast
                        ps_r = ps_rpool.tile([P, CH], F32, tag="ps_r")
                        nc.tensor.matmul(ps_r, r(ones[0:1, :]), r(d_sb),
                                         start=True, stop=True)
                        rb = asbpool.tile([P, CH], F32, tag="rb")
                        nc.vector.reciprocal(out=rb, in_=ps_r)
                        ao_sb = asbpool.tile([P, CH], F32, tag="ao_sb")
                        nc.vector.tensor_mul(ao_sb, ps_o, rb)
                        dma(ao_d[hh * P:(hh + 1) * P, c0:c0 + CH], ao_sb)

            # =============== Phase 4: O projection ===========================
            with (
                tc.tile_pool(name="aoc", bufs=2) as aocpool,
                tc.tile_pool(name="wo", bufs=2) as wopool,
                tc.tile_pool(name="aop", bufs=1) as aoppool,
                tc.tile_pool(name="oo", bufs=3) as oopool,
                tc.tile_pool(name="po", bufs=4, space="PSUM") as po,
                tc.tile_pool(name="pzo", bufs=1, space="PSUM") as pzop,
            ):
                ato_sb = aoppool.tile([R, D], F32, tag="aTo")
                dma(ato_sb, aT["o"][:, :], f32r=True)

                for c0 in range(0, QTOK, CH):
                    aoc = aocpool.tile([P, NT, CH], F32, tag="aoc")
                    dma(aoc, ao_d[:, c0:c0 + CH].rearrange("(n p) f -> p n f", p=P),
                        f32r=True)
                    # z_o for this chunk
                    ps = pzop.tile([R, CH], F32, tag="pzo")
                    for di in range(NT):
                        nc.tensor.matmul(ps, r(bTall[:, 3, di, :]), r(aoc[:, di, :]),
                                         start=(di == 0), stop=(di == NT - 1))
                    nc.vector.tensor_scalar_mul(r(zo[:, c0:c0 + CH]), ps, SCALING)

                    for do in range(NT):
                        wo_sb = wopool.tile([P, NT, P], F32, tag="wo")
                        dma(wo_sb, wT["o"][:, do * P:(do + 1) * P]
                            .rearrange("(n p) f -> p n f", p=P), f32r=True)
                        ps = po.tile([P, CH], F32, tag="po")
                        for ki in range(NT):
                            nc.tensor.matmul(ps, r(wo_sb[:, ki, :]), r(aoc[:, ki, :]),
                                             start=(ki == 0), stop=False)
                        nc.tensor.matmul(ps, r(ato_sb[:, do * P:(do + 1) * P]),
                                         r(zo[:, c0:c0 + CH]),
                                         start=False, stop=True)
                        o_sb = oopool.tile([P, CH], F32, tag="oo")
                        nc.vector.tensor_scalar_add(o_sb, ps, biasall[:, 3, do:do + 1])
                        dma(yt[do * P:(do + 1) * P, c0:c0 + CH], o_sb)

    nc.compile()
    return nc


def kernel(**inputs):
    inp = {k: np.asarray(v, dtype=np.float32) for k, v in inputs.items()}
    x = inp["x"]

    if "nc" not in _cache:
        _cache["nc"] = _build()
    nc = _cache["nc"]

    shared = {}
    for p in "qkvo":
        shared[f"w{p}T"] = np.ascontiguousarray(inp[f"W{p}"].T)
        shared[f"b{p}"] = inp[f"b{p}"]
        shared[f"a{p}T"] = np.ascontiguousarray(inp[f"A{p}"].T)
        shared[f"bt{p}"] = np.ascontiguousarray(inp[f"B{p}"].T)

    in_maps = []
    for c in range(NCORES):
        b, hf = c // 2, c % 2
        # permute tokens so this core's query tokens are rows 0..QTOK-1
        xbv = np.concatenate([x[b, hf * QTOK:(hf + 1) * QTOK],
                              x[b, (1 - hf) * QTOK:(2 - hf) * QTOK]])
        m = dict(shared)
        m["xb"] = np.ascontiguousarray(xbv)
        in_maps.append(m)

    trace = bool(int(os.environ.get("KERNEL_TRACE", "0")))
    res = run_bass_kernel_spmd(nc, in_maps, list(range(NCORES)), trace=trace)
    _cache["last_exec_time_ns"] = res.exec_time_ns
    _cache["last_result"] = res

    y = np.empty((B, L, D), dtype=np.float32)
    for c in range(NCORES):
        b, hf = c // 2, c % 2
        y[b, hf * QTOK:(hf + 1) * QTOK, :] = res.results[c]["yt"].T
    return y

